# revision 13
# baseline (speedup 1.0000x reference)
"""MoE (top-2 of 8 experts, SwiGLU) on 8 Trainium2 NeuronCores.

Strategy: tensor-parallel x expert-parallel (TP2 x EP4). Each core runs TWO
expert slots, each covering HALF the inter dimension (1024 of 2048) of one
expert; an expert's two halves land on two different cores. Slot sizes are
fixed SPMD shapes c1 >= c2; the host pairs the heaviest expert with the
lightest so per-core work ~ c1+c2 ~ (max big + max small) instead of
2*max_e: ~4% less PE time than pure expert-parallel, with identical weight
DMA volume (half of two experts == all of one).

  - Host computes the router (tiny: [2048,1024]@[1024,8]) and the top-2
    dispatch; packs each core's inputs (its two expert-halves' weights +
    their tokens). This IS the sharding step.
  - Core, per slot s with c_s tokens over inter-half rows:
        hT = w1h[s] @ x_sT           (gate/up fused halves, [2048, c_s])
        yT = silu(hT_gate) * hT_up   ([1024, c_s], bf16)
        oT = (w2h[s] @ yT) * combine ([1024, c_s] PARTIAL over inter)
    GEMMs in bf16, fp32 PSUM accumulation; fp8 was measured on-HW at only
    2x MACs (DoubleRow) which cannot beat bf16 under the 2e-2 error gate
    (plain fp8 rel-err = 5.2e-2).
  - Host scatter-ADDS the 16 partial outputs back to token order; the two
    halves of each expert sum naturally.

Layouts keep tokens on the PSUM free dim everywhere so no on-device
transposes are needed; weights are pre-transposed on the host.
"""

import sys

sys.path.insert(0, "/opt/trn_rl_repo")

import numpy as np
import ml_dtypes

import concourse.bass as bass  # noqa: F401  (bass must import before tile)
import concourse.tile as tile
from concourse import bacc, mybir
from concourse.bass_utils import run_bass_kernel_spmd

T = 2048
H = 1024
INTER = 2048
E = 8
TOPK = 2
N_CORES = 8
P = 128
IH = INTER // 2        # inter-half per slot

DT = mybir.dt.bfloat16
NP_DT = ml_dtypes.bfloat16

_PROGRAM_CACHE = {}    # (c1, c2) -> compiled Bacc program

KH = H // P            # 8  k-tiles for GEMM1 (contract over H)
KI = IH // P           # 8  k-tiles for GEMM2 (contract over inter-half)
NPAIR = IH // P        # 8  gate/up pairs per slot
NH = H // P            # 8  output h-tiles
NW1P = 8               # w1 column pieces (512 cols each)
W1PC = 2 * INTER // NW1P


def _route(x, router_w):
    """Replicates the reference router in fp32 numpy.

    Returns per-expert (token_indices, combine_weights)."""
    gating = (x @ router_w.T).astype(np.float32)              # [T, E]
    m = gating.max(axis=1, keepdims=True)
    p = np.exp(gating - m, dtype=np.float32)
    probs = p / p.sum(axis=1, keepdims=True)
    order = np.argsort(-probs, axis=1, kind="stable")         # ties -> lower idx
    sel = order[:, :TOPK]                                     # [T, K]
    topw = np.take_along_axis(probs, sel, axis=1)             # [T, K]

    idxs, wts = [], []
    for e in range(E):
        m_e = sel == e                                        # [T, K]
        rows = np.nonzero(m_e.any(axis=1))[0]
        idxs.append(rows.astype(np.int64))
        wts.append(topw[m_e].astype(np.float32))              # aligned with rows
    return idxs, wts


def _chunks(c):
    """Split c tokens into near-equal chunks of <=512 (PSUM bank limit)."""
    n = -(-c // 512)
    base = -(-(-(-c // n)) // 4) * 4                          # ceil(c/n) to mult of 4
    sizes = []
    left = c
    for _ in range(n - 1):
        sizes.append(base)
        left -= base
    sizes.append(left)
    return [s for s in sizes if s > 0]


def _plan(idxs):
    """Pair heavy experts with light ones; assign each expert's two
    inter-halves to the two cores its pair occupies.

    Returns (c1, c2, cores) where cores[i] = ((expA, halfA), (expB, halfB));
    slot shapes c1 (big) and c2 (small) are shared SPMD constants."""
    loads = [len(ix) for ix in idxs]
    order = sorted(range(E), key=lambda e: -loads[e])
    big, small = order[:E // 2], list(reversed(order[E // 2:]))
    cores = []
    for i in range(E // 2):
        a, b = big[i], small[i]
        cores.append(((a, 0), (b, 0)))
        cores.append(((a, 1), (b, 1)))
    c1 = max(4, -(-max(loads[e] for e in big) // 4) * 4)
    c2 = max(4, -(-max(loads[e] for e in small) // 4) * 4)
    return c1, c2, cores


def _pack_inputs(x, w1, w2, idxs, wts, c1, c2, cores):
    """Per-core input dict for the SPMD program.

    xt   [H, c1+c2]    bf16: slot-A tokens in cols [0:c1), slot-B in [c1:).
    w1t  [H, 2*INTER]  bf16 columns = [Agate | Aup | Bgate | Bup] halves.
    w2t  [INTER, H]    bf16 rows = [A w2-half.T | B w2-half.T].
    scale[P, c1+c2]    f32 combine weights broadcast over partitions.
    """
    xt_f32 = x.T
    in_maps = []
    for (ea, ha), (eb, hb) in cores:
        ct = c1 + c2
        xt = np.zeros((H, ct), dtype=NP_DT)
        sc = np.zeros((P, ct), dtype=np.float32)
        w1t = np.empty((H, 2 * INTER), dtype=NP_DT)
        w2t = np.empty((INTER, H), dtype=NP_DT)
        for s, (e, h, off, cs) in enumerate(
                ((ea, ha, 0, c1), (eb, hb, c1, c2))):
            n = len(idxs[e])
            xt[:, off:off + n] = xt_f32[:, idxs[e]].astype(NP_DT)
            sc[:, off:off + n] = wts[e][None, :]
            gsl = slice(h * IH, (h + 1) * IH)                  # gate rows
            usl = slice(INTER + h * IH, INTER + (h + 1) * IH)  # up rows
            w1t[:, s * INTER: s * INTER + IH] = \
                w1[e, gsl, :].T.astype(NP_DT)
            w1t[:, s * INTER + IH: (s + 1) * INTER] = \
                w1[e, usl, :].T.astype(NP_DT)
            w2t[s * IH:(s + 1) * IH, :] = \
                w2[e][:, h * IH:(h + 1) * IH].T.astype(NP_DT)
        in_maps.append({"xt": xt, "w1t": np.ascontiguousarray(w1t),
                        "w2t": np.ascontiguousarray(w2t), "scale": sc})
    return in_maps


def _build_program(c1, c2, loop_n=0, unroll=1, warm_pre=True):
    """One SPMD program: two expert-half MLP slots over c1 / c2 tokens.

    loop_n > 0 wraps the body in an on-device For_i loop (perf harness
    only; the graded path uses loop_n=0 = straight-line body). unroll
    repeats the body per iteration: bodies inside one iteration have no
    For_i all-engine barrier between them, so body i+1's input DMAs
    overlap body i's GEMM2 tail and the slope measures the steady-state
    marginal cost of one body. warm_pre places the PE warmup before the
    loop (it only helps the cold start; iterations keep the PE hot)."""
    nc = bacc.Bacc("TRN2", target_bir_lowering=False, debug=False,
                   num_devices=N_CORES)
    f32 = mybir.dt.float32
    ct = c1 + c2
    xt_d = nc.dram_tensor("xt", [H, ct], DT, kind="ExternalInput").ap()
    w1t_d = nc.dram_tensor("w1t", [H, 2 * INTER], DT, kind="ExternalInput").ap()
    w2t_d = nc.dram_tensor("w2t", [INTER, H], DT, kind="ExternalInput").ap()
    sc_d = nc.dram_tensor("scale", [P, ct], f32, kind="ExternalInput").ap()
    out_d = nc.dram_tensor("out", [H, ct], f32, kind="ExternalOutput").ap()

    from contextlib import ExitStack
    with tile.TileContext(nc) as tc, ExitStack() as ctx:
        wpool = ctx.enter_context(tc.tile_pool(name="weights", bufs=1))
        xpool = ctx.enter_context(tc.tile_pool(name="xt", bufs=1))
        ypool = ctx.enter_context(tc.tile_pool(name="yt", bufs=1))
        apool = ctx.enter_context(tc.tile_pool(name="act", bufs=2))
        opool = ctx.enter_context(tc.tile_pool(name="ot", bufs=2))
        pgpool = ctx.enter_context(tc.tile_pool(name="psg", bufs=3, space="PSUM"))
        pupool = ctx.enter_context(tc.tile_pool(name="psu", bufs=3, space="PSUM"))
        popool = ctx.enter_context(tc.tile_pool(name="pso", bufs=2, space="PSUM"))

        def emit_warmup():
            # ~3.5 us of dependency-free matmuls on an (uninitialized)
            # scratch tile: the PE HAM clock-gate warms to 2.4 GHz during the
            # initial DMA wait instead of throttling the first real matmuls.
            warm_sb = xpool.tile([P, P], DT, tag="warm")
            nc.vector.memset(warm_sb[:, 0:1], 0.0)
            ps_w = popool.tile([P, P], f32, tag="pso", name="ps_warm")
            # 44 matmuls: ~32 burn the HAM cold window (1.2 GHz, ~107 ns
            # each) on garbage, the rest bridge until the first real
            # operands land (~3.9 us).
            for _ in range(44):
                nc.tensor.matmul(ps_w[:], lhsT=warm_sb[:], rhs=warm_sb[:],
                                 start=True, stop=True)

        if warm_pre:
            emit_warmup()

        if loop_n:
            ctx.enter_context(tc.For_i(
                0, loop_n, 1,
                hint_engines=(mybir.EngineType.PE, mybir.EngineType.SP,
                              mybir.EngineType.Activation, mybir.EngineType.DVE)))

        if not warm_pre:
            emit_warmup()

        for _body in range(unroll):
            _emit_body(nc, c1, c2, xt_d, w1t_d, w2t_d, sc_d, out_d,
                       wpool, xpool, ypool, apool, opool,
                       pgpool, pupool, popool)

    nc.compile()
    return nc


def _emit_body(nc, c1, c2, xt_d, w1t_d, w2t_d, sc_d, out_d,
               wpool, xpool, ypool, apool, opool, pgpool, pupool, popool):
    f32 = mybir.dt.float32
    ct = c1 + c2
    # slot s: token cols [off, off+cs), w1 pieces [4s, 4s+4), w2 rows half s
    slots = []
    for s, (off, cs) in enumerate(((0, c1), (c1, c2))):
        csls = []
        c0 = off
        for cn in _chunks(cs):
            csls.append((slice(c0, c0 + cn), cn))
            c0 += cn
        slots.append((s, off, cs, csls))

    # ---- input loads ----
    # One merged DMA per logical tensor/piece: the HWDGE prep cost is
    # per-instruction (~625 ns, serialized), so many small DMAs stall the
    # PE at startup.

    # xt: slot-A chunk-1 columns first, split by k — they gate the first
    # matmuls of the body.
    xt_t = xpool.tile([P, KH, ct], DT, tag="xt")
    xt_view = xt_d.rearrange("(k p) c -> p k c", p=P)
    c1a = slots[0][3][0][1]  # first chunk width of slot A
    nc.sync.dma_start(out=xt_t[:, :KH // 2, :c1a],
                      in_=xt_view[:, :KH // 2, :c1a])
    xt_sb = [xt_t[:, k, :] for k in range(KH)]

    # first 256 cols of w1 (slot A gate piece 0 head) for k=0..3
    w1_0a = wpool.tile([P, KH, 2 * P], DT, tag="w1_0a")
    w1_0a_view = w1t_d[:, :2 * P].rearrange("(k p) c -> p k c", p=P)
    nc.sync.dma_start(out=w1_0a[:, :KH // 2, :],
                      in_=w1_0a_view[:, :KH // 2, :])

    w1_t = {}

    def load_w1_cols(lo, hi, tag):
        t = wpool.tile([P, KH, hi - lo], DT, tag=tag, name=tag)
        nc.sync.dma_start(
            out=t[:], in_=w1t_d[:, lo:hi].rearrange("(k p) c -> p k c", p=P))
        return t

    nc.sync.dma_start(out=xt_t[:, KH // 2:, :c1a],
                      in_=xt_view[:, KH // 2:, :c1a])
    nc.sync.dma_start(out=w1_0a[:, KH // 2:, :],
                      in_=w1_0a_view[:, KH // 2:, :])
    w1_t["0a"] = w1_0a
    w1_t["0b"] = load_w1_cols(2 * P, W1PC, "w1_0b")
    # xt rest split at the slot boundary: slot A's chunk 2 (small) lands
    # before the first quad finishes chunk 1; slot B's columns trail piece 3
    # (they are not read until slot B's GEMM1, ~2/5 into the body).
    if c1a < c1:
        nc.sync.dma_start(out=xt_t[:, :, c1a:c1], in_=xt_view[:, :, c1a:c1])
    # piece order follows PE consumption: slot A gate {0,1} with up {2,3}
    # quad-paired, then slot B gate {4,5} / up {6,7}.
    for piece in (2, 1, 3):
        w1_t[piece] = load_w1_cols(piece * W1PC, (piece + 1) * W1PC,
                                   f"w1_{piece}")
    nc.sync.dma_start(out=xt_t[:, :, c1:], in_=xt_view[:, :, c1:])
    for piece in (4, 6, 5, 7):
        w1_t[piece] = load_w1_cols(piece * W1PC, (piece + 1) * W1PC,
                                   f"w1_{piece}")

    # w2t: two merged DMAs (one per slot, 8 k-tiles each)
    w2_sb = []
    for half in range(2):
        t = wpool.tile([P, KI, H], DT, tag=f"w2_{half}")
        rs = slice(half * IH, (half + 1) * IH)
        nc.sync.dma_start(
            out=t[:], in_=w2t_d[rs, :].rearrange("(k p) c -> p k c", p=P))
        w2_sb.append(t)

    sc_sb = xpool.tile([P, ct], f32, tag="sc")
    nc.sync.dma_start(out=sc_sb[:], in_=sc_d[:])

    def w1_slice(s, k, i):
        # stationary lhsT [P(h), P(inter)]; i in [0, 2*NPAIR): gate tiles
        # 0..7, up tiles 8..15 of slot s. Global piece = 4*s + i//4.
        piece, sub = 4 * s + i // 4, i % 4
        if piece == 0:
            if sub < 2:
                return w1_t["0a"][:, k, P * sub:P * (sub + 1)]
            return w1_t["0b"][:, k, P * (sub - 2):P * (sub - 1)]
        return w1_t[piece][:, k, P * sub:P * (sub + 1)]

    # ---- GEMM1 + SwiGLU for both slots: yT[i] = silu(gate_i) * up_i ----
    # Chunk loop innermost so each w1 stationary tile is consumed across
    # the full GEMM1 span. Quad structure (4 gate pairs, then their 4
    # ups) keeps ~8 us of PE work queued ahead of each w1 piece DMA.
    # Both GEMM1s run before either GEMM2 so each slot's DVE mul tail is
    # absorbed by the other slot's PE work instead of stalling its GEMM2.
    yts = {}
    for s, off, cs, csls in slots:
        yt_sb = [None] * NPAIR
        for q in range(NPAIR // 4):
            quad = range(4 * q, 4 * q + 4)
            sgs = {}
            for i in quad:
                yt_sb[i] = ypool.tile([P, cs], DT, tag=f"yt{s}_{i}",
                                      name=f"yt{s}_{i}")
            for ci, (csl, cn) in enumerate(csls):
                for i in quad:
                    ps_g = pgpool.tile([P, cn], f32, tag="psg")
                    for k in range(KH):
                        nc.tensor.matmul(ps_g[:], lhsT=w1_slice(s, k, i),
                                         rhs=xt_sb[k][:, csl],
                                         start=(k == 0), stop=(k == KH - 1))
                    sg = apool.tile([P, cn], f32, tag=f"sg{i % 4}_{ci}")
                    nc.scalar.activation(sg[:], ps_g[:],
                                         mybir.ActivationFunctionType.Silu)
                    sgs[(i, ci)] = sg
            for ci, (csl, cn) in enumerate(csls):
                for i in quad:
                    ps_u = pupool.tile([P, cn], f32, tag="psu")
                    for k in range(KH):
                        nc.tensor.matmul(ps_u[:],
                                         lhsT=w1_slice(s, k, i + NPAIR),
                                         rhs=xt_sb[k][:, csl],
                                         start=(k == 0), stop=(k == KH - 1))
                    nc.vector.tensor_mul(
                        yt_sb[i][:, csl.start - off:csl.stop - off],
                        sgs[(i, ci)][:], ps_u[:])
        yts[s] = yt_sb

    # ---- GEMM2 + combine scale (partial over each inter-half) ----
    # One merged out DMA per (slot, j): 16 HWDGE preps per body instead of
    # 32 — the serialized ~625 ns preps otherwise outrun the 14 us per-slot
    # GEMM2 span and stall the PE on ot-buffer reuse.
    for s, off, cs, csls in slots:
        yt_sb = yts[s]
        for j in range(NH):
            ot = opool.tile([P, cs], f32, tag=f"ot{s}", name=f"ot{s}_{j}")
            for csl, cn in csls:
                ps_o = popool.tile([P, cn], f32, tag="pso")
                for k in range(KI):
                    nc.tensor.matmul(
                        ps_o[:],
                        lhsT=w2_sb[s][:, k, P * j:P * (j + 1)],
                        rhs=yt_sb[k][:, csl.start - off:csl.stop - off],
                        start=(k == 0), stop=(k == KI - 1))
                nc.vector.tensor_mul(ot[:, csl.start - off:csl.stop - off],
                                     sc_sb[:, csl], ps_o[:])
            nc.sync.dma_start(out=out_d[P * j:P * (j + 1), off:off + cs],
                              in_=ot[:])


def kernel(hidden_states, w1, w2, router_w):
    x = np.ascontiguousarray(np.asarray(hidden_states, dtype=np.float32)
                             .reshape(T, H))
    w1 = np.asarray(w1, dtype=np.float32)
    w2 = np.asarray(w2, dtype=np.float32)
    router_w = np.asarray(router_w, dtype=np.float32)

    idxs, wts = _route(x, router_w)
    c1, c2, cores = _plan(idxs)

    nc = _PROGRAM_CACHE.get((c1, c2))
    if nc is None:
        nc = _PROGRAM_CACHE[(c1, c2)] = _build_program(c1, c2)

    in_maps = _pack_inputs(x, w1, w2, idxs, wts, c1, c2, cores)

    try:
        res = run_bass_kernel_spmd(nc, in_maps, list(range(N_CORES)))
    except Exception:
        # transient runtime hiccups (e.g. mesh desync on a fresh session)
        # usually clear on retry
        res = run_bass_kernel_spmd(nc, in_maps, list(range(N_CORES)))

    out = np.zeros((T, H), dtype=np.float32)
    for core, ((ea, _), (eb, _)) in enumerate(cores):
        o = res.results[core]["out"]
        for e, off in ((ea, 0), (eb, c1)):
            n = len(idxs[e])
            if n:
                out[idxs[e]] += o[:, off:off + n].T
    return out.reshape(1, T, H)
